# revision 8
# baseline (speedup 1.0000x reference)
"""Trainium2 raw-Bass kernel for nn_InteractionPruningLayer (sparse_attention).

Math (B=1024, F=256, D=64):
    qkv   = einsum('fd,nde->nfe', indicator, W_qkv)            # [3,F,D]
    gate  = (trans[0] @ trans[1].T > 0);  G = (qkv1 @ qkv0.T) * gate
    s[n,b,f] = feature[b,f,:] . qkv[n,f,:];  t = s0*s2;  u = s1
    out[b,i,:] = t[b,i] * sum_j u[b,j] * G[i,j] * qkv2[j,:]

Split of work:
    host   — weight prep (G, qkv2) and the per-(b,f) projections s/t/u
             (~1.7% of FLOPs, done in f32; the axon tunnel at ~50MB/s
             makes shipping the 67MB feature the bottleneck, while t/u
             are 2MB)
    device — 8 cores, batch-parallel, 128 rows each: the O(B*F^2*D)
             gated interaction contraction (~98% of FLOPs) producing
             the full [B,F,D] output:
                 K2[j,(i,d)] = G[i,j] * qkv2[j,d]          (built on-chip)
                 inner[b,(i,d)] = sum_j uT[j,b] * K2[j,(i,d)]
                 out[b,(i,d)] = t[b,i] * inner[b,(i,d)]    (stored bf16)

Raw bass blocks + explicit semaphores (Tile-emitted multi-wait sync does
not codegen under this walrus build). All bulk wire tensors are bf16.
The bf16 output travels as two uint8 byte-planes (sign+exp plane
zstd-compresses to ~0.44 on the axon wire, mantissa plane doesn't);
the host recombines them losslessly and upcasts to f32. A persistent
jax compilation cache avoids the ~0.7s/call re-lowering the fresh
jax.jit inside run_bass_via_pjrt would otherwise pay.
"""

import numpy as np
import ml_dtypes

B, F, D = 1024, 256, 64
NCORES = 8
BL = B // NCORES
FD = F * D                 # 16384
NCH = 16                   # main-mm chunks of 1024
_compiled = None


def _setup_jax_cache():
    import jax
    try:
        if jax.config.jax_compilation_cache_dir is None:
            jax.config.update("jax_compilation_cache_dir",
                              "/tmp/bass_jax_cache")
            jax.config.update("jax_persistent_cache_min_entry_size_bytes", -1)
            jax.config.update("jax_persistent_cache_min_compile_time_secs", 0)
    except Exception:
        pass


def _host_precompute(indicator, W_qk, W_qkv):
    """consts [128, 640] bf16: GT (2x[128,256] halves) + qkv2 (2x[128,64]).
    Also returns qkv [3,F,D] f32 for the host-side s projections."""
    ind = indicator.astype(np.float32)
    qkv = np.einsum('fd,nde->nfe', ind, W_qkv.astype(np.float32))
    trans = np.einsum('fd,nde->nfe', ind, W_qk.astype(np.float32))
    gate = (trans[0] @ trans[1].T) > 0
    G = np.where(gate, qkv[1] @ qkv[0].T, np.float32(0.0)).astype(np.float32)
    GT = np.ascontiguousarray(G.T)                       # [j, i]
    consts = np.zeros((128, 640), dtype=np.float32)
    consts[:, 0:256] = GT[0:128]
    consts[:, 256:512] = GT[128:256]
    consts[:, 512:576] = qkv[2][0:128]
    consts[:, 576:640] = qkv[2][128:256]
    return consts.astype(ml_dtypes.bfloat16), qkv


def _host_tu(feature, qkv):
    """t = s0*s2 (f32 [B,F]), uT packed per-core [128,(jc,b)] bf16."""
    f = np.asarray(feature, dtype=np.float32)
    s = np.einsum('bfd,nfd->nbf', f, qkv, optimize=True)
    t = (s[0] * s[2]).astype(np.float32)                 # [B, F]
    u = s[1].astype(ml_dtypes.bfloat16)                  # [B, F]
    # uT[core][j_local, jc*128 + b] = u[core*128 + b, jc*128 + j_local]
    uT = np.ascontiguousarray(
        u.reshape(NCORES, 128, 2, 128).transpose(0, 3, 2, 1)
        .reshape(NCORES, 128, 256))
    return t, uT


def _build_bass():
    import concourse.bass as bass
    from concourse import mybir

    nc = bass.Bass()
    f32, bf16, u8 = mybir.dt.float32, mybir.dt.bfloat16, mybir.dt.uint8

    const_d = nc.declare_dram_parameter("consts", [128, 640], bf16, isOutput=False)
    t_d = nc.declare_dram_parameter("tvec", [128, 256], f32, isOutput=False)
    u_d = nc.declare_dram_parameter("uT", [128, 256], bf16, isOutput=False)
    ohi_d = nc.declare_dram_parameter("out_hi", [BL, FD], u8, isOutput=True)
    olo_d = nc.declare_dram_parameter("out_lo", [BL, FD], u8, isOutput=True)

    consts = nc.alloc_sbuf_tensor("consts_sb", [128, 640], bf16).ap()
    k2 = nc.alloc_sbuf_tensor("k2", [128, 2 * FD], bf16).ap()   # [j, (jc,i,d)]
    grep = nc.alloc_sbuf_tensor("grep", [128, 2, 4096], bf16).ap()
    t_sb = nc.alloc_sbuf_tensor("t_sb", [128, 256], f32).ap()
    uT = nc.alloc_sbuf_tensor("uT_sb", [128, 2, 128], bf16).ap()
    osb = nc.alloc_sbuf_tensor("osb", [128, FD], bf16).ap()
    ohi = nc.alloc_sbuf_tensor("ohi", [128, FD], u8).ap()
    olo = nc.alloc_sbuf_tensor("olo", [128, FD], u8).ap()
    mp = [nc.alloc_psum_tensor(f"mp{i}", [128, 1024], f32).ap() for i in range(2)]
    # little-endian byte view of osb: [..., 0] = mantissa byte, [..., 1] = sign+exp
    ob3 = osb.bitcast(u8).rearrange("b (x two) -> b x two", two=2)

    gt = consts[:, 0:512].rearrange("k (c i) -> k c i", c=2)
    qkv2 = consts[:, 512:640].rearrange("k (c d) -> k c d", c=2)
    k2q = k2.rearrange("k (q x) -> k q x", q=8)
    t3 = t_sb.rearrange("b (i x) -> b i x", x=1)

    with (
        nc.Block() as block,
        nc.semaphore("sL") as sL,
        nc.semaphore("sA") as sA,
        nc.semaphore("sK") as sK,
        nc.semaphore("sM") as sM,
        nc.semaphore("sE") as sE,
        nc.semaphore("sO") as sO,
    ):
        @block.gpsimd
        def _(g):
            g.dma_start(out=consts[:], in_=const_d[:]).then_inc(sL, 16)
            g.dma_start(out=t_sb[:], in_=t_d[:]).then_inc(sL, 16)
            g.dma_start(out=uT[:], in_=u_d.rearrange("k (c b) -> k c b", c=2)
                        ).then_inc(sL, 16)
            for c in range(4):
                g.wait_ge(sE, 4 * (c + 1))
                g.dma_start(out=ohi_d[:, 4096 * c:4096 * (c + 1)],
                            in_=ohi[:, 4096 * c:4096 * (c + 1)]).then_inc(sO, 16)
                g.dma_start(out=olo_d[:, 4096 * c:4096 * (c + 1)],
                            in_=olo[:, 4096 * c:4096 * (c + 1)]).then_inc(sO, 16)
            g.wait_ge(sO, 128)

        @block.tensor
        def _(t):
            t.wait_ge(sL, 48)
            t.wait_ge(sK, 8)
            for k in range(NCH):
                if k >= 2:
                    t.wait_ge(sE, k - 1)
                c0 = 1024 * k
                for jc in range(2):
                    for h in range(2):
                        mm = t.matmul(
                            out=mp[k % 2][:, 512 * h:512 * (h + 1)],
                            lhsT=uT[:, jc, :],
                            rhs=k2[:, jc * FD + c0 + 512 * h:
                                   jc * FD + c0 + 512 * (h + 1)],
                            start=(jc == 0), stop=(jc == 1))
                mm.then_inc(sM, 1)

        @block.vector
        def _(v):
            v.wait_ge(sL, 48)
            for q in range(8):
                v.wait_ge(sA, q + 1)
                jc = q // 4
                v.tensor_mul(
                    k2q[:, q, :].rearrange("k (i d) -> k i d", d=D),
                    grep[:, q % 2, :].rearrange("k (i d) -> k i d", d=D),
                    qkv2[:, jc, :].unsqueeze(1).broadcast_to([128, 64, D]),
                ).then_inc(sK, 1)
            for k in range(NCH):
                v.wait_ge(sM, k + 1)
                c0 = 1024 * k
                v.tensor_mul(
                    osb[:, c0:c0 + 1024].rearrange("b (i d) -> b i d", d=D),
                    mp[k % 2].rearrange("b (i d) -> b i d", d=D),
                    t3[:, 16 * k:16 * (k + 1), :].broadcast_to([128, 16, D]),
                )
                v.tensor_copy(ohi[:, c0:c0 + 1024], ob3[:, c0:c0 + 1024, 1])
                v.tensor_copy(olo[:, c0:c0 + 1024],
                              ob3[:, c0:c0 + 1024, 0]).then_inc(sE, 1)

        @block.scalar
        def _(a):
            a.wait_ge(sL, 16)
            for q in range(8):
                if q >= 2:
                    a.wait_ge(sK, q - 1)
                jc, ih = q // 4, q % 4
                a.copy(out=grep[:, q % 2, :].rearrange("k (i d) -> k i d", d=D),
                       in_=gt[:, jc, 64 * ih:64 * (ih + 1)]
                       .unsqueeze(2).broadcast_to([128, 64, D]))
                a.copy(out=grep[0:1, q % 2, 0:1],
                       in_=grep[0:1, q % 2, 0:1]).then_inc(sA, 1)

    return nc


def _make_in_maps(feature, consts, qkv):
    t, uT = _host_tu(feature, qkv)
    return [{"consts": consts,
             "tvec": np.ascontiguousarray(t[c * BL:(c + 1) * BL]),
             "uT": uT[c]} for c in range(NCORES)]


def kernel(feature, indicator, W_qk, W_qkv):
    global _compiled
    _setup_jax_cache()
    from concourse.bass_utils import run_bass_kernel_spmd

    consts, qkv = _host_precompute(indicator, W_qk, W_qkv)
    if _compiled is None:
        _compiled = _build_bass()
    nc = _compiled

    in_maps = _make_in_maps(feature, consts, qkv)
    res = run_bass_kernel_spmd(nc, in_maps, list(range(NCORES)))
    out = np.concatenate([_recombine(r) for r in res.results], axis=0)
    return out


def _recombine(r):
    w = (r["out_hi"].astype(np.uint16) << 8) | r["out_lo"].astype(np.uint16)
    return w.view(ml_dtypes.bfloat16).astype(np.float32).reshape(BL, F, D)


# revision 11
# speedup vs baseline: 1.2609x; 1.2609x over previous
"""Trainium2 raw-Bass kernel for nn_InteractionPruningLayer (sparse_attention).

Math (B=1024, F=256, D=64):
    qkv   = einsum('fd,nde->nfe', indicator, W_qkv)            # [3,F,D]
    gate  = (trans[0] @ trans[1].T > 0);  G = (qkv1 @ qkv0.T) * gate
    s[n,b,f] = feature[b,f,:] . qkv[n,f,:];  t = s0*s2;  u = s1
    out[b,i,:] = t[b,i] * sum_j u[b,j] * G[i,j] * qkv2[j,:]

Split of work:
    host   — weight prep (G, qkv2) and the per-(b,f) projections s/t/u
             (~1.7% of FLOPs, done in f32; the axon tunnel at ~50MB/s
             makes shipping the 67MB feature the bottleneck, while t/u
             are 2MB)
    device — 8 cores, batch-parallel, 128 rows each: the O(B*F^2*D)
             gated interaction contraction (~98% of FLOPs) producing
             the full [B,F,D] output:
                 K2[j,(i,d)] = G[i,j] * qkv2[j,d]          (built on-chip)
                 inner[b,(i,d)] = sum_j uT[j,b] * K2[j,(i,d)]
                 out[b,(i,d)] = t[b,i] * inner[b,(i,d)]    (stored bf16)

Raw bass blocks + explicit semaphores (Tile-emitted multi-wait sync does
not codegen under this walrus build). All bulk wire tensors are bf16.
The bf16 output travels as two uint8 byte-planes (sign+exp plane
zstd-compresses to ~0.44 on the axon wire, mantissa plane doesn't);
the host recombines them losslessly and upcasts to f32. A persistent
jax compilation cache avoids the ~0.7s/call re-lowering the fresh
jax.jit inside run_bass_via_pjrt would otherwise pay.
"""

import numpy as np
import ml_dtypes

B, F, D = 1024, 256, 64
NCORES = 8
BL = B // NCORES
FD = F * D                 # 16384
NCH = 16                   # main-mm chunks of 1024
_compiled = None


def _setup_jax_cache():
    import jax
    try:
        if jax.config.jax_compilation_cache_dir is None:
            jax.config.update("jax_compilation_cache_dir",
                              "/tmp/bass_jax_cache")
            jax.config.update("jax_persistent_cache_min_entry_size_bytes", -1)
            jax.config.update("jax_persistent_cache_min_compile_time_secs", 0)
    except Exception:
        pass


def _host_precompute(indicator, W_qk, W_qkv):
    """consts [128, 640] bf16: GT (2x[128,256] halves) + qkv2 (2x[128,64]).
    Also returns qkv [3,F,D] f32 for the host-side s projections."""
    ind = indicator.astype(np.float32)
    qkv = np.einsum('fd,nde->nfe', ind, W_qkv.astype(np.float32))
    trans = np.einsum('fd,nde->nfe', ind, W_qk.astype(np.float32))
    gate = (trans[0] @ trans[1].T) > 0
    G = np.where(gate, qkv[1] @ qkv[0].T, np.float32(0.0)).astype(np.float32)
    GT = np.ascontiguousarray(G.T)                       # [j, i]
    consts = np.zeros((128, 640), dtype=np.float32)
    consts[:, 0:256] = GT[0:128]
    consts[:, 256:512] = GT[128:256]
    consts[:, 512:576] = qkv[2][0:128]
    consts[:, 576:640] = qkv[2][128:256]
    return consts.astype(ml_dtypes.bfloat16), qkv


def _host_tu(feature, qkv):
    """t = s0*s2 (f32 [B,F]), uT packed per-core [128,(jc,b)] bf16."""
    f = np.asarray(feature, dtype=np.float32)
    s = np.einsum('bfd,nfd->nbf', f, qkv, optimize=True)
    t = (s[0] * s[2]).astype(np.float32)                 # [B, F]
    u = s[1].astype(ml_dtypes.bfloat16)                  # [B, F]
    # uT[core][j_local, jc*128 + b] = u[core*128 + b, jc*128 + j_local]
    uT = np.ascontiguousarray(
        u.reshape(NCORES, 128, 2, 128).transpose(0, 3, 2, 1)
        .reshape(NCORES, 128, 256))
    return t, uT


def _build_bass():
    import concourse.bass as bass
    from concourse import mybir

    nc = bass.Bass()
    f32, bf16, u8 = mybir.dt.float32, mybir.dt.bfloat16, mybir.dt.uint8

    const_d = nc.declare_dram_parameter("consts", [128, 640], bf16, isOutput=False)
    t_d = nc.declare_dram_parameter("tvec", [128, 256], f32, isOutput=False)
    u_d = nc.declare_dram_parameter("uT", [128, 256], bf16, isOutput=False)
    out_d = nc.declare_dram_parameter("out", [2, BL, FD], u8, isOutput=True)

    consts = nc.alloc_sbuf_tensor("consts_sb", [128, 640], bf16).ap()
    k2 = nc.alloc_sbuf_tensor("k2", [128, 2 * FD], bf16).ap()   # [j, (jc,i,d)]
    grep = nc.alloc_sbuf_tensor("grep", [128, 2, 4096], bf16).ap()
    t_sb = nc.alloc_sbuf_tensor("t_sb", [128, 256], f32).ap()
    uT = nc.alloc_sbuf_tensor("uT_sb", [128, 2, 128], bf16).ap()
    osb = nc.alloc_sbuf_tensor("osb", [128, FD], bf16).ap()
    ohi = nc.alloc_sbuf_tensor("ohi", [128, FD], u8).ap()
    olo = nc.alloc_sbuf_tensor("olo", [128, FD], u8).ap()
    mp = [nc.alloc_psum_tensor(f"mp{i}", [128, 1024], f32).ap() for i in range(2)]
    # little-endian byte view of osb: [..., 0] = mantissa byte, [..., 1] = sign+exp
    ob3 = osb.bitcast(u8).rearrange("b (x two) -> b x two", two=2)

    gt = consts[:, 0:512].rearrange("k (c i) -> k c i", c=2)
    qkv2 = consts[:, 512:640].rearrange("k (c d) -> k c d", c=2)
    k2q = k2.rearrange("k (q x) -> k q x", q=8)
    t3 = t_sb.rearrange("b (i x) -> b i x", x=1)

    with (
        nc.Block() as block,
        nc.semaphore("sL") as sL,
        nc.semaphore("sA") as sA,
        nc.semaphore("sK") as sK,
        nc.semaphore("sM") as sM,
        nc.semaphore("sE") as sE,
        nc.semaphore("sO") as sO,
    ):
        @block.gpsimd
        def _(g):
            g.dma_start(out=consts[:], in_=const_d[:]).then_inc(sL, 16)
            g.dma_start(out=t_sb[:], in_=t_d[:]).then_inc(sL, 16)
            g.dma_start(out=uT[:], in_=u_d.rearrange("k (c b) -> k c b", c=2)
                        ).then_inc(sL, 16)
            for c in range(4):
                g.wait_ge(sE, 4 * (c + 1))
                g.dma_start(out=out_d[0, :, 4096 * c:4096 * (c + 1)],
                            in_=ohi[:, 4096 * c:4096 * (c + 1)]).then_inc(sO, 16)
                g.dma_start(out=out_d[1, :, 4096 * c:4096 * (c + 1)],
                            in_=olo[:, 4096 * c:4096 * (c + 1)]).then_inc(sO, 16)
            g.wait_ge(sO, 128)

        @block.tensor
        def _(t):
            t.wait_ge(sL, 48)
            t.wait_ge(sK, 8)
            for k in range(NCH):
                if k >= 2:
                    t.wait_ge(sE, k - 1)
                c0 = 1024 * k
                for jc in range(2):
                    for h in range(2):
                        mm = t.matmul(
                            out=mp[k % 2][:, 512 * h:512 * (h + 1)],
                            lhsT=uT[:, jc, :],
                            rhs=k2[:, jc * FD + c0 + 512 * h:
                                   jc * FD + c0 + 512 * (h + 1)],
                            start=(jc == 0), stop=(jc == 1))
                mm.then_inc(sM, 1)

        @block.vector
        def _(v):
            v.wait_ge(sL, 48)
            for q in range(8):
                v.wait_ge(sA, q + 1)
                jc = q // 4
                v.tensor_mul(
                    k2q[:, q, :].rearrange("k (i d) -> k i d", d=D),
                    grep[:, q % 2, :].rearrange("k (i d) -> k i d", d=D),
                    qkv2[:, jc, :].unsqueeze(1).broadcast_to([128, 64, D]),
                ).then_inc(sK, 1)
            for k in range(NCH):
                v.wait_ge(sM, k + 1)
                c0 = 1024 * k
                v.tensor_mul(
                    osb[:, c0:c0 + 1024].rearrange("b (i d) -> b i d", d=D),
                    mp[k % 2].rearrange("b (i d) -> b i d", d=D),
                    t3[:, 16 * k:16 * (k + 1), :].broadcast_to([128, 16, D]),
                )
                v.tensor_copy(ohi[:, c0:c0 + 1024], ob3[:, c0:c0 + 1024, 1])
                v.tensor_copy(olo[:, c0:c0 + 1024],
                              ob3[:, c0:c0 + 1024, 0]).then_inc(sE, 1)

        @block.scalar
        def _(a):
            a.wait_ge(sL, 16)
            for q in range(8):
                if q >= 2:
                    a.wait_ge(sK, q - 1)
                jc, ih = q // 4, q % 4
                a.copy(out=grep[:, q % 2, :].rearrange("k (i d) -> k i d", d=D),
                       in_=gt[:, jc, 64 * ih:64 * (ih + 1)]
                       .unsqueeze(2).broadcast_to([128, 64, D]))
                a.copy(out=grep[0:1, q % 2, 0:1],
                       in_=grep[0:1, q % 2, 0:1]).then_inc(sA, 1)

    return nc


def _make_in_maps(feature, consts, qkv):
    t, uT = _host_tu(feature, qkv)
    return [{"consts": consts,
             "tvec": np.ascontiguousarray(t[c * BL:(c + 1) * BL]),
             "uT": uT[c]} for c in range(NCORES)]


def kernel(feature, indicator, W_qk, W_qkv):
    global _compiled
    _setup_jax_cache()
    from concourse.bass_utils import run_bass_kernel_spmd

    consts, qkv = _host_precompute(indicator, W_qk, W_qkv)
    if _compiled is None:
        _compiled = _build_bass()
    nc = _compiled

    in_maps = _make_in_maps(feature, consts, qkv)
    res = run_bass_kernel_spmd(nc, in_maps, list(range(NCORES)))
    out = np.concatenate([_recombine(r) for r in res.results], axis=0)
    return out


def _recombine(r):
    o = r["out"]
    w = (o[0].astype(np.uint16) << 8) | o[1].astype(np.uint16)
    return w.view(ml_dtypes.bfloat16).astype(np.float32).reshape(BL, F, D)


# revision 13
# speedup vs baseline: 1.5450x; 1.2253x over previous
"""Trainium2 raw-Bass kernel for nn_InteractionPruningLayer (sparse_attention).

Math (B=1024, F=256, D=64):
    qkv   = einsum('fd,nde->nfe', indicator, W_qkv)            # [3,F,D]
    gate  = (trans[0] @ trans[1].T > 0);  G = (qkv1 @ qkv0.T) * gate
    s[n,b,f] = feature[b,f,:] . qkv[n,f,:];  t = s0*s2;  u = s1
    out[b,i,:] = t[b,i] * sum_j u[b,j] * G[i,j] * qkv2[j,:]

Split of work:
    host   — weight prep (G, qkv2) and the per-(b,f) projections s/t/u
             (~1.7% of FLOPs, done in f32; the axon tunnel at ~50MB/s
             makes shipping the 67MB feature the bottleneck, while t/u
             are 2MB)
    device — 8 cores, batch-parallel, 128 rows each: the O(B*F^2*D)
             gated interaction contraction (~98% of FLOPs) producing
             the full [B,F,D] output:
                 K2[j,(i,d)] = G[i,j] * qkv2[j,d]          (built on-chip)
                 inner[b,(i,d)] = sum_j uT[j,b] * K2[j,(i,d)]
                 out[b,(i,d)] = t[b,i] * inner[b,(i,d)]    (stored bf16)

Raw bass blocks + explicit semaphores (Tile-emitted multi-wait sync does
not codegen under this walrus build). All bulk wire tensors are bf16.
The bf16 output is upcast to f32 on the host. A persistent
jax compilation cache avoids the ~0.7s/call re-lowering the fresh
jax.jit inside run_bass_via_pjrt would otherwise pay.
"""

import numpy as np
import ml_dtypes

B, F, D = 1024, 256, 64
NCORES = 8
BL = B // NCORES
FD = F * D                 # 16384
NCH = 16                   # main-mm chunks of 1024
_compiled = None


def _setup_jax_cache():
    import jax
    try:
        if jax.config.jax_compilation_cache_dir is None:
            jax.config.update("jax_compilation_cache_dir",
                              "/tmp/bass_jax_cache")
            jax.config.update("jax_persistent_cache_min_entry_size_bytes", -1)
            jax.config.update("jax_persistent_cache_min_compile_time_secs", 0)
    except Exception:
        pass


def _host_precompute(indicator, W_qk, W_qkv):
    """consts [128, 640] bf16: GT (2x[128,256] halves) + qkv2 (2x[128,64]).
    Also returns qkv [3,F,D] f32 for the host-side s projections."""
    indicator = np.asarray(indicator)
    W_qk = np.asarray(W_qk)
    W_qkv = np.asarray(W_qkv)
    ind = indicator.astype(np.float32)
    qkv = np.einsum('fd,nde->nfe', ind, W_qkv.astype(np.float32))
    trans = np.einsum('fd,nde->nfe', ind, W_qk.astype(np.float32))
    gate = (trans[0] @ trans[1].T) > 0
    G = np.where(gate, qkv[1] @ qkv[0].T, np.float32(0.0)).astype(np.float32)
    GT = np.ascontiguousarray(G.T)                       # [j, i]
    consts = np.zeros((128, 640), dtype=np.float32)
    consts[:, 0:256] = GT[0:128]
    consts[:, 256:512] = GT[128:256]
    consts[:, 512:576] = qkv[2][0:128]
    consts[:, 576:640] = qkv[2][128:256]
    return consts.astype(ml_dtypes.bfloat16), qkv


def _host_tu(feature, qkv):
    """t = s0*s2 (f32 [B,F]), uT packed per-core [128,(jc,b)] bf16."""
    f = np.asarray(feature, dtype=np.float32)
    s = np.einsum('bfd,nfd->nbf', f, qkv, optimize=True)
    t = (s[0] * s[2]).astype(np.float32)                 # [B, F]
    u = s[1].astype(ml_dtypes.bfloat16)                  # [B, F]
    # uT[core][j_local, jc*128 + b] = u[core*128 + b, jc*128 + j_local]
    uT = np.ascontiguousarray(
        u.reshape(NCORES, 128, 2, 128).transpose(0, 3, 2, 1)
        .reshape(NCORES, 128, 256))
    return t, uT


def _build_bass():
    import concourse.bass as bass
    from concourse import mybir

    nc = bass.Bass()
    f32, bf16, u8 = mybir.dt.float32, mybir.dt.bfloat16, mybir.dt.uint8

    const_d = nc.declare_dram_parameter("consts", [128, 640], bf16, isOutput=False)
    t_d = nc.declare_dram_parameter("tvec", [128, 256], f32, isOutput=False)
    u_d = nc.declare_dram_parameter("uT", [128, 256], bf16, isOutput=False)
    out_d = nc.declare_dram_parameter("out", [BL, FD], bf16, isOutput=True)

    consts = nc.alloc_sbuf_tensor("consts_sb", [128, 640], bf16).ap()
    k2 = nc.alloc_sbuf_tensor("k2", [128, 2 * FD], bf16).ap()   # [j, (jc,i,d)]
    grep = nc.alloc_sbuf_tensor("grep", [128, 2, 4096], bf16).ap()
    t_sb = nc.alloc_sbuf_tensor("t_sb", [128, 256], f32).ap()
    uT = nc.alloc_sbuf_tensor("uT_sb", [128, 2, 128], bf16).ap()
    osb = nc.alloc_sbuf_tensor("osb", [128, FD], bf16).ap()
    mp = [nc.alloc_psum_tensor(f"mp{i}", [128, 1024], f32).ap() for i in range(2)]

    gt = consts[:, 0:512].rearrange("k (c i) -> k c i", c=2)
    qkv2 = consts[:, 512:640].rearrange("k (c d) -> k c d", c=2)
    k2q = k2.rearrange("k (q x) -> k q x", q=8)
    t3 = t_sb.rearrange("b (i x) -> b i x", x=1)

    with (
        nc.Block() as block,
        nc.semaphore("sL") as sL,
        nc.semaphore("sA") as sA,
        nc.semaphore("sK") as sK,
        nc.semaphore("sM") as sM,
        nc.semaphore("sE") as sE,
        nc.semaphore("sO") as sO,
    ):
        @block.gpsimd
        def _(g):
            g.dma_start(out=consts[:], in_=const_d[:]).then_inc(sL, 16)
            g.dma_start(out=t_sb[:], in_=t_d[:]).then_inc(sL, 16)
            g.dma_start(out=uT[:], in_=u_d.rearrange("k (c b) -> k c b", c=2)
                        ).then_inc(sL, 16)
            for c in range(4):
                g.wait_ge(sE, 4 * (c + 1))
                g.dma_start(out=out_d[:, 4096 * c:4096 * (c + 1)],
                            in_=osb[:, 4096 * c:4096 * (c + 1)]).then_inc(sO, 16)
            g.wait_ge(sO, 64)

        @block.tensor
        def _(t):
            t.wait_ge(sL, 48)
            t.wait_ge(sK, 8)
            for k in range(NCH):
                if k >= 2:
                    t.wait_ge(sE, k - 1)
                c0 = 1024 * k
                for jc in range(2):
                    for h in range(2):
                        mm = t.matmul(
                            out=mp[k % 2][:, 512 * h:512 * (h + 1)],
                            lhsT=uT[:, jc, :],
                            rhs=k2[:, jc * FD + c0 + 512 * h:
                                   jc * FD + c0 + 512 * (h + 1)],
                            start=(jc == 0), stop=(jc == 1))
                mm.then_inc(sM, 1)

        @block.vector
        def _(v):
            v.wait_ge(sL, 48)
            for q in range(8):
                v.wait_ge(sA, q + 1)
                jc = q // 4
                v.tensor_mul(
                    k2q[:, q, :].rearrange("k (i d) -> k i d", d=D),
                    grep[:, q % 2, :].rearrange("k (i d) -> k i d", d=D),
                    qkv2[:, jc, :].unsqueeze(1).broadcast_to([128, 64, D]),
                ).then_inc(sK, 1)
            for k in range(NCH):
                v.wait_ge(sM, k + 1)
                c0 = 1024 * k
                v.tensor_mul(
                    osb[:, c0:c0 + 1024].rearrange("b (i d) -> b i d", d=D),
                    mp[k % 2].rearrange("b (i d) -> b i d", d=D),
                    t3[:, 16 * k:16 * (k + 1), :].broadcast_to([128, 16, D]),
                ).then_inc(sE, 1)

        @block.scalar
        def _(a):
            a.wait_ge(sL, 16)
            for q in range(8):
                if q >= 2:
                    a.wait_ge(sK, q - 1)
                jc, ih = q // 4, q % 4
                a.copy(out=grep[:, q % 2, :].rearrange("k (i d) -> k i d", d=D),
                       in_=gt[:, jc, 64 * ih:64 * (ih + 1)]
                       .unsqueeze(2).broadcast_to([128, 64, D]))
                a.copy(out=grep[0:1, q % 2, 0:1],
                       in_=grep[0:1, q % 2, 0:1]).then_inc(sA, 1)

    return nc


def _make_in_maps(feature, consts, qkv):
    t, uT = _host_tu(feature, qkv)
    return [{"consts": consts,
             "tvec": np.ascontiguousarray(t[c * BL:(c + 1) * BL]),
             "uT": uT[c]} for c in range(NCORES)]


def kernel(feature, indicator, W_qk, W_qkv):
    global _compiled
    _setup_jax_cache()
    from concourse.bass_utils import run_bass_kernel_spmd

    consts, qkv = _host_precompute(indicator, W_qk, W_qkv)
    if _compiled is None:
        _compiled = _build_bass()
    nc = _compiled

    in_maps = _make_in_maps(feature, consts, qkv)
    res = run_bass_kernel_spmd(nc, in_maps, list(range(NCORES)))
    out = np.concatenate([_recombine(r) for r in res.results], axis=0)
    return out


def _recombine(r):
    return r["out"].astype(np.float32).reshape(BL, F, D)


# revision 34
# speedup vs baseline: 1.5747x; 1.0193x over previous
"""Trainium2 raw-Bass kernel for nn_InteractionPruningLayer (sparse_attention).

Math (B=1024, F=256, D=64):
    qkv   = einsum('fd,nde->nfe', indicator, W_qkv)            # [3,F,D]
    gate  = (trans[0] @ trans[1].T > 0);  G = (qkv1 @ qkv0.T) * gate
    s[n,b,f] = feature[b,f,:] . qkv[n,f,:];  t = s0*s2;  u = s1
    out[b,i,:] = t[b,i] * sum_j u[b,j] * G[i,j] * qkv2[j,:]

Split of work:
    host   — weight prep (G, qkv2) and the per-(b,f) projections s/t/u
             (~1.7% of FLOPs, done in f32; the axon tunnel at ~50MB/s
             makes shipping the 67MB feature the bottleneck, while t/u
             are 2MB)
    device — 8 cores, batch-parallel, 128 rows each: the O(B*F^2*D)
             gated interaction contraction (~98% of FLOPs) producing
             the full [B,F,D] output:
                 K2[j,(i,d)] = G[i,j] * qkv2[j,d]          (built on-chip)
                 inner[b,(i,d)] = sum_j uT[j,b] * K2[j,(i,d)]
                 out[b,(i,d)] = t[b,i] * inner[b,(i,d)]    (stored bf16)

Raw bass blocks + explicit semaphores (Tile-emitted multi-wait sync does
not codegen under this walrus build). All bulk wire tensors are bf16;
the bf16 output is upcast to f32 on the host. A persistent jax
compilation cache avoids the ~0.7s/call re-lowering the fresh jax.jit
inside run_bass_via_pjrt would otherwise pay.
"""

import numpy as np
import ml_dtypes

B, F, D = 1024, 256, 64
NCORES = 8
BL = B // NCORES
FD = F * D                 # 16384
NCH = 16                   # main-mm chunks of 1024
_compiled = None


def _setup_jax_cache():
    import jax
    try:
        if jax.config.jax_compilation_cache_dir is None:
            jax.config.update("jax_compilation_cache_dir",
                              "/tmp/bass_jax_cache")
            jax.config.update("jax_persistent_cache_min_entry_size_bytes", -1)
            jax.config.update("jax_persistent_cache_min_compile_time_secs", 0)
    except Exception:
        pass


def _host_precompute(indicator, W_qk, W_qkv):
    """consts [128, 640] bf16: GT (2x[128,256] halves) + qkv2 (2x[128,64]).
    Also returns qkv [3,F,D] f32 for the host-side s projections."""
    indicator = np.asarray(indicator)
    W_qk = np.asarray(W_qk)
    W_qkv = np.asarray(W_qkv)
    ind = indicator.astype(np.float32)
    qkv = np.einsum('fd,nde->nfe', ind, W_qkv.astype(np.float32))
    trans = np.einsum('fd,nde->nfe', ind, W_qk.astype(np.float32))
    gate = (trans[0] @ trans[1].T) > 0
    G = np.where(gate, qkv[1] @ qkv[0].T, np.float32(0.0)).astype(np.float32)
    GT = np.ascontiguousarray(G.T)                       # [j, i]
    consts = np.zeros((128, 640), dtype=np.float32)
    consts[:, 0:256] = GT[0:128]
    consts[:, 256:512] = GT[128:256]
    consts[:, 512:576] = qkv[2][0:128]
    consts[:, 576:640] = qkv[2][128:256]
    return consts.astype(ml_dtypes.bfloat16), qkv


def _host_tu(feature, qkv):
    """t = s0*s2 (f32 [B,F]), uT packed per-core [128,(jc,b)] bf16."""
    f = np.asarray(feature, dtype=np.float32)
    s = np.einsum('bfd,nfd->nbf', f, qkv, optimize=True)
    t = (s[0] * s[2]).astype(np.float32)                 # [B, F]
    u = s[1].astype(ml_dtypes.bfloat16)                  # [B, F]
    # uT[core][j_local, jc*128 + b] = u[core*128 + b, jc*128 + j_local]
    uT = np.ascontiguousarray(
        u.reshape(NCORES, 128, 2, 128).transpose(0, 3, 2, 1)
        .reshape(NCORES, 128, 256))
    return t, uT


def _build_bass():
    import concourse.bass as bass
    from concourse import mybir

    nc = bass.Bass()
    f32, bf16 = mybir.dt.float32, mybir.dt.bfloat16

    const_d = nc.declare_dram_parameter("consts", [128, 640], bf16, isOutput=False)
    t_d = nc.declare_dram_parameter("tvec", [128, 256], f32, isOutput=False)
    u_d = nc.declare_dram_parameter("uT", [128, 256], bf16, isOutput=False)
    out_d = nc.declare_dram_parameter("out", [BL, FD], bf16, isOutput=True)

    consts = nc.alloc_sbuf_tensor("consts_sb", [128, 640], bf16).ap()
    k2 = nc.alloc_sbuf_tensor("k2", [128, 2 * FD], bf16).ap()   # [j, (jc,i,d)]
    grep = nc.alloc_sbuf_tensor("grep", [128, 2, 4096], bf16).ap()
    t_sb = nc.alloc_sbuf_tensor("t_sb", [128, 256], f32).ap()
    uT = nc.alloc_sbuf_tensor("uT_sb", [128, 2, 128], bf16).ap()
    osb = nc.alloc_sbuf_tensor("osb", [128, FD], bf16).ap()
    mp = [nc.alloc_psum_tensor(f"mp{i}", [128, 1024], f32).ap() for i in range(2)]

    gt = consts[:, 0:512].rearrange("k (c i) -> k c i", c=2)
    qkv2 = consts[:, 512:640].rearrange("k (c d) -> k c d", c=2)
    k2q = k2.rearrange("k (q x) -> k q x", q=8)
    t3 = t_sb.rearrange("b (i x) -> b i x", x=1)

    with (
        nc.Block() as block,
        nc.semaphore("sL") as sL,
        nc.semaphore("sA") as sA,
        nc.semaphore("sK") as sK,
        nc.semaphore("sM") as sM,
        nc.semaphore("sE") as sE,
        nc.semaphore("sO") as sO,
    ):
        # Semaphore state can survive across executions on these
        # long-lived axon terminals (alloc does NOT clear). Each engine
        # zeroes the sems it waits on first; gpsimd delays its first DMA
        # (~40us of NOPs) so no increment can precede the clears.
        @block.gpsimd
        def _(g):
            g.sem_clear(sE)
            g.sem_clear(sO)
            for _ in range(2):
                g.nop(cycle_cnt=30000)
            g.dma_start(out=consts[:], in_=const_d[:]).then_inc(sL, 16)
            g.dma_start(out=t_sb[:], in_=t_d[:]).then_inc(sL, 16)
            g.dma_start(out=uT[:], in_=u_d.rearrange("k (c b) -> k c b", c=2)
                        ).then_inc(sL, 16)
            for c in range(4):
                g.wait_ge(sE, 4 * (c + 1))
                g.dma_start(out=out_d[:, 4096 * c:4096 * (c + 1)],
                            in_=osb[:, 4096 * c:4096 * (c + 1)]).then_inc(sO, 16)
            g.wait_ge(sO, 64)

        @block.tensor
        def _(t):
            t.sem_clear(sL)
            t.sem_clear(sK)
            t.sem_clear(sE)
            t.wait_ge(sL, 48)
            t.wait_ge(sK, 8)
            for k in range(NCH):
                if k >= 2:
                    t.wait_ge(sE, k - 1)
                c0 = 1024 * k
                for jc in range(2):
                    for h in range(2):
                        mm = t.matmul(
                            out=mp[k % 2][:, 512 * h:512 * (h + 1)],
                            lhsT=uT[:, jc, :],
                            rhs=k2[:, jc * FD + c0 + 512 * h:
                                   jc * FD + c0 + 512 * (h + 1)],
                            start=(jc == 0), stop=(jc == 1))
                mm.then_inc(sM, 1)

        @block.vector
        def _(v):
            v.sem_clear(sL)
            v.sem_clear(sA)
            v.sem_clear(sM)
            v.wait_ge(sL, 48)
            for q in range(8):
                v.wait_ge(sA, q + 1)
                jc = q // 4
                v.tensor_mul(
                    k2q[:, q, :].rearrange("k (i d) -> k i d", d=D),
                    grep[:, q % 2, :].rearrange("k (i d) -> k i d", d=D),
                    qkv2[:, jc, :].unsqueeze(1).broadcast_to([128, 64, D]),
                ).then_inc(sK, 1)
            for k in range(NCH):
                v.wait_ge(sM, k + 1)
                c0 = 1024 * k
                v.tensor_mul(
                    osb[:, c0:c0 + 1024].rearrange("b (i d) -> b i d", d=D),
                    mp[k % 2].rearrange("b (i d) -> b i d", d=D),
                    t3[:, 16 * k:16 * (k + 1), :].broadcast_to([128, 16, D]),
                ).then_inc(sE, 1)

        @block.scalar
        def _(a):
            a.sem_clear(sL)
            a.sem_clear(sK)
            a.wait_ge(sL, 16)
            for q in range(8):
                if q >= 2:
                    a.wait_ge(sK, q - 1)
                jc, ih = q // 4, q % 4
                a.copy(out=grep[:, q % 2, :].rearrange("k (i d) -> k i d", d=D),
                       in_=gt[:, jc, 64 * ih:64 * (ih + 1)]
                       .unsqueeze(2).broadcast_to([128, 64, D]))
                a.copy(out=grep[0:1, q % 2, 0:1],
                       in_=grep[0:1, q % 2, 0:1]).then_inc(sA, 1)

    return nc


def _make_in_maps(feature, consts, qkv):
    t, uT = _host_tu(feature, qkv)
    in_maps = [{"consts": consts,
                "tvec": np.ascontiguousarray(t[c * BL:(c + 1) * BL]),
                "uT": uT[c]} for c in range(NCORES)]
    return in_maps, t


def kernel(feature, indicator, W_qk, W_qkv):
    global _compiled
    _setup_jax_cache()
    from concourse.bass_utils import run_bass_kernel_spmd

    consts, qkv = _host_precompute(indicator, W_qk, W_qkv)
    if _compiled is None:
        _compiled = _build_bass()
    nc = _compiled

    in_maps, _t = _make_in_maps(feature, consts, qkv)
    res = run_bass_kernel_spmd(nc, in_maps, list(range(NCORES)))
    out = np.concatenate(
        [r["out"].astype(np.float32).reshape(BL, F, D) for r in res.results],
        axis=0)
    return out
